# revision 31
# baseline (speedup 1.0000x reference)
"""Trainium2 Bass kernel for nn_Decoder_46660524704357.

Reference computation (shapes hardcoded in DEFAULT_CFG):
    B, C, L, D, E, K = 64, 23, 26000, 64, 512, 3
    eos  = eos_emb @ eos_W.T + eos_b          # [B,C,D]
    bin_emb = emb_table[bin_ids]              # [C,L,D]
    a = bin_emb @ Wb.T                        # [C,L,K]   Wb = fc_W[:, :D]
    e = eos @ We.T + fc_b                     # [B,C,K]   We = fc_W[:, D:]
    out = relu(a[None,:,:,:] + e[:,:,None,:]) # [B,C,L,K]

Sharding: split L across the 8 cores (Lc = 3250 each).  Each core:
  - computes the eos projection chain (tiny) to get e[B,C,K] on-device,
  - for each chromosome c and each output tile, runs ONE fused matmul:
        out[p=(k*B+b), l] = sum_d Wsel[d, p] * embT[d, l] + e_row[p] * 1
    where Wsel[d, k*B+b] = fc_W[k, d] (constant) and the (D+1)-th contract
    row of embT is all-ones so the e term rides along.  ScalarE applies ReLU
    on the PSUM->SBUF copy, DMA writes a [B, C, K, Lc] layout output.
Host re-interleaves K innermost at the end.
"""

import numpy as np

DEFAULT_CFG = dict(B=64, C=23, L=26000, D=64, E=512, K=3, NCORES=8)

_CACHE = {}


def _derived(cfg):
    B, C, L, D, E, K, NCORES = (cfg[k] for k in ("B", "C", "L", "D", "E", "K", "NCORES"))
    d = dict(cfg)
    d["LC"] = L // NCORES
    d["BC"] = B * C
    d["EP"] = min(128, E)              # contract chunk for eos matmul
    assert E % d["EP"] == 0
    d["NQ"] = E // d["EP"]
    d["ROWS"] = K * B                  # output partition rows (b*K + k)
    # partition tiles over ROWS: cut at b boundaries so each tile's DMA rows
    # merge into contiguous [K*LC] runs per b
    tiles = []
    bmax = 128 // K                    # b's per tile
    b0 = 0
    while b0 < B:
        nb = min(bmax, B - b0)
        tiles.append((b0 * K, nb * K, b0, nb))
        b0 += nb
    d["PTILES"] = tiles                # (p_off, p_n, b0, nb)
    fc = min(512, d["LC"])
    d["NF"] = [fc] * (d["LC"] // fc) + ([d["LC"] % fc] if d["LC"] % fc else [])
    return d


def _build_nc(cfg=None, selector_f32r=True, eos_f32r=False):
    import concourse.bass as bass  # noqa: F401
    import concourse.mybir as mybir
    import concourse.tile as tile
    from concourse import bacc

    g = _derived(cfg or DEFAULT_CFG)
    B, C, D, K = g["B"], g["C"], g["D"], g["K"]
    LC, BC, EP, NQ, ROWS = g["LC"], g["BC"], g["EP"], g["NQ"], g["ROWS"]
    FCH = min(512, BC)

    f32 = mybir.dt.float32
    f32r = mybir.dt.float32r
    # dtype for tensors consumed by the big selector matmul: fp32r streams
    # 1 col/cycle (vs 4 for fp32).  The BIR verifier requires the whole
    # producer chain to carry the f32r dtype.
    fsel = f32r if selector_f32r else f32
    feos = f32r if eos_f32r else f32

    # Bacc (not plain Bass): its compile() passes split multi-sem waits and
    # move matmul waits to ldweights — required for walrus codegen.
    nc = bacc.Bacc(None)

    embT = nc.declare_dram_parameter("embT", [D + 1, C * LC], fsel, isOutput=False)
    eosE = nc.declare_dram_parameter("eosE", [EP, NQ * BC], feos, isOutput=False)
    eosW = nc.declare_dram_parameter("eosW", [EP, NQ * D], feos, isOutput=False)
    WeT = nc.declare_dram_parameter("WeT", [D, K], feos, isOutput=False)
    eos_b = nc.declare_dram_parameter("eos_b", [D, 1], f32, isOutput=False)
    fc_b = nc.declare_dram_parameter("fc_b", [1, K], f32, isOutput=False)
    wsel = nc.declare_dram_parameter("wsel", [D, C * ROWS], fsel, isOutput=False)
    out = nc.declare_dram_parameter("out", [B, C, K, LC], f32, isOutput=True)

    with tile.TileContext(nc) as tc:
        with (
            tc.tile_pool(name="consts", bufs=1) as consts,
            tc.tile_pool(name="setup_sb", bufs=1) as setup_sb,
            tc.tile_pool(name="setup_ps", bufs=1, space="PSUM") as setup_ps,
            tc.tile_pool(name="emb", bufs=3) as emb_pool,
            tc.tile_pool(name="osb", bufs=5) as osb_pool,
            tc.tile_pool(name="ops", bufs=6, space="PSUM") as ops_pool,
        ):
            # ---- constants / setup -------------------------------------
            # all setup loads ride gpsimd's SWDGE (16-way engine fan-out);
            # eosE is chunked so the eos matmuls start on the first chunk
            se = consts.tile([D + 1, C * ROWS], fsel)        # selector weights
            nc.gpsimd.dma_start(se[0:D, :], wsel[:, :])

            eosE_sb = setup_sb.tile([EP, NQ * BC], feos)
            for q in range(NQ):
                nc.gpsimd.dma_start(
                    eosE_sb[:, q * BC:(q + 1) * BC],
                    eosE[:, q * BC:(q + 1) * BC])
            eosW_sb = setup_sb.tile([EP, NQ * D], feos)
            nc.gpsimd.dma_start(eosW_sb[:, :], eosW[:, :])
            WeT_sb = setup_sb.tile([D, K], f32)
            nc.sync.dma_start(WeT_sb[:, :], WeT[:, :])
            eosb_sb = setup_sb.tile([D, 1], f32)
            nc.sync.dma_start(eosb_sb[:, :], eos_b[:, :])
            fcb_sb = setup_sb.tile([1, K], f32)
            nc.sync.dma_start(fcb_sb[:, :], fc_b[:, :])

            # eosT[d, bc] = sum_E eos_W[d, E] * eos_emb[bc, E]  (+ eos_b)
            eosT_sb = setup_sb.tile([D, BC], feos)
            bc_chunks = [(i, min(FCH, BC - i)) for i in range(0, BC, FCH)]
            for bc0, nbc in bc_chunks:
                eosT_ps = setup_ps.tile([D, nbc], f32, tag="eos_ps")
                for q in range(NQ):
                    nc.tensor.matmul(
                        eosT_ps[:, :],
                        lhsT=eosW_sb[:, q * D:(q + 1) * D],
                        rhs=eosE_sb[:, q * BC + bc0: q * BC + bc0 + nbc],
                        start=(q == 0),
                        stop=(q == NQ - 1),
                    )
                nc.scalar.add(eosT_sb[:, bc0:bc0 + nbc], eosT_ps[:, :], eosb_sb[:, 0:1])

            # e_row[p=(c,b,k)] = sum_d We[k,d]*eosT[d,bc] + fc_b[k], computed
            # directly in selector-column order:
            #   X[d, (c,b,k)] = eosT[d, b*C+c] * WeT[d, k]   (DVE, bcast APs)
            #   X[0, :]      += fc_b[k]                       (DVE)
            #   e_row         = colsum(X)                     (PE, ones lhsT)
            X = setup_sb.tile([D, C * ROWS], f32)
            eosT_g = eosT_sb[:, :].rearrange("d (b c) -> d c b", b=B, c=C) \
                .unsqueeze(3).broadcast_to([D, C, B, K])
            We_g = WeT_sb[:, :].unsqueeze(1).unsqueeze(1).broadcast_to([D, C, B, K])
            X_w = X[:, :].rearrange("d (c b k) -> d c b k", c=C, b=B, k=K)
            nc.vector.tensor_mul(X_w, eosT_g, We_g)
            fcb_g = fcb_sb[:, :].unsqueeze(1).unsqueeze(1).broadcast_to([1, C, B, K])
            X0_w = X[0:1, :].rearrange("d (c b k) -> d c b k", c=C, b=B, k=K)
            nc.vector.tensor_add(X0_w, X0_w.copy(), fcb_g)

            ones64 = setup_sb.tile([D, 1], f32)
            nc.vector.memset(ones64[:, :], 1.0)
            row_chunks = [(i, min(512, C * ROWS - i)) for i in range(0, C * ROWS, 512)]
            for r0, nr in row_chunks:
                e_ps = setup_ps.tile([D + 1, nr], f32, tag="eos_ps")
                nc.tensor.matmul(
                    e_ps[D:D + 1, :],
                    lhsT=ones64[:, 0:1],
                    rhs=X[:, r0:r0 + nr],
                    start=True,
                    stop=True,
                )
                nc.scalar.activation(
                    se[D:D + 1, r0:r0 + nr], e_ps[D:D + 1, :],
                    mybir.ActivationFunctionType.Copy,
                )

            # ---- main loop ---------------------------------------------
            # DMA engine assignment: each engine's dynamic-HW queue is a
            # separate hardware ring, and each dma_start trigger costs ~1us
            # on the issuing engine — so spread big transfers across
            # engines and keep the count low (one out-DMA per (c, ptile)).
            out_bkl = out.rearrange("b c k l -> c b (k l)")
            # weighted round-robin: gpsimd's software DGE spreads descriptors
            # across all 16 DMA engines; the sync/scalar hardware queues only
            # reach 2-3 engines each
            out_engines = [nc.gpsimd, nc.gpsimd, nc.sync, nc.gpsimd, nc.gpsimd, nc.scalar]
            n_dma = 0
            for c in range(C):
                # et via gpsimd SWDGE: one DMA's descriptors fan out across
                # all 16 DMA engines (HW queues only reach 2-3), so the load
                # latency is ~16x lower — keeps PE fed
                et = emb_pool.tile([D + 1, LC], fsel, tag="embT")
                nc.gpsimd.dma_start(et[:, :], embT[:, c * LC:(c + 1) * LC])
                for ti, (p_off, p_n, b0, nb) in enumerate(g["PTILES"]):
                    so = osb_pool.tile([p_n, LC], f32, tag="out_sb")
                    f0 = 0
                    for nf in g["NF"]:
                        po = ops_pool.tile([p_n, nf], f32, tag="out_ps")
                        nc.tensor.matmul(
                            po[:, :],
                            lhsT=se[:, c * ROWS + p_off: c * ROWS + p_off + p_n],
                            rhs=et[:, f0:f0 + nf],
                            start=True,
                            stop=True,
                        )
                        # ReLU on PSUM->SBUF copy; alternate engines so the
                        # scalar engine isn't the serial resource
                        if ti % 2 == 0:
                            nc.scalar.activation(
                                so[:, f0:f0 + nf], po[:, :],
                                mybir.ActivationFunctionType.Relu,
                            )
                        else:
                            nc.vector.tensor_scalar_max(so[:, f0:f0 + nf], po[:, :], 0.0)
                        f0 += nf
                    out_engines[n_dma % len(out_engines)].dma_start(
                        out_bkl[c, b0:b0 + nb, :], so[:, :]
                    )
                    n_dma += 1
    nc.finalize()
    return nc


def _host_prep(eos_emb, bin_ids, emb_table, eos_W, eos_b, fc_W, fc_b, cfg=None):
    """Build the per-core input maps."""
    g = _derived(cfg or DEFAULT_CFG)
    B, C, L, D, E, K = g["B"], g["C"], g["L"], g["D"], g["E"], g["K"]
    NCORES, LC, BC, EP, NQ, ROWS = (
        g["NCORES"], g["LC"], g["BC"], g["EP"], g["NQ"], g["ROWS"])

    eos_emb = np.ascontiguousarray(eos_emb, dtype=np.float32)
    emb_table = np.ascontiguousarray(emb_table, dtype=np.float32)
    bin_ids = np.asarray(bin_ids)

    # gather (identity when bin_ids == arange, which is the spec'd fill)
    V = C * L
    flat_ids = bin_ids.reshape(-1)
    if flat_ids.shape[0] == V and emb_table.shape[0] == V and \
            flat_ids[0] == 0 and flat_ids[-1] == V - 1 and \
            np.array_equal(flat_ids, np.arange(V, dtype=flat_ids.dtype)):
        bin_emb = emb_table.reshape(C, L, D)
    else:
        bin_emb = emb_table[bin_ids.reshape(C, L)]

    eosE = np.ascontiguousarray(
        eos_emb.reshape(BC, E).T.reshape(NQ, EP, BC).transpose(1, 0, 2).reshape(EP, NQ * BC)
    )
    eosW = np.ascontiguousarray(
        np.asarray(eos_W, np.float32).T.reshape(NQ, EP, D).transpose(1, 0, 2).reshape(EP, NQ * D)
    )
    fc_W = np.asarray(fc_W, np.float32)
    WeT = np.ascontiguousarray(fc_W[:, D:].T)            # [D, K]
    eos_b_in = np.asarray(eos_b, np.float32).reshape(D, 1)
    fc_b_in = np.asarray(fc_b, np.float32).reshape(1, K)
    # wsel[d, c*ROWS + b*K + k] = fc_W[k, d]
    wsel1 = np.tile(fc_W[:, :D], (B, 1)).T               # [D, B*K] (b-major)
    wsel = np.ascontiguousarray(np.tile(wsel1, (1, C)))  # [D, C*ROWS]

    shared = dict(eosE=eosE, eosW=eosW, WeT=WeT, eos_b=eos_b_in, fc_b=fc_b_in, wsel=wsel)

    in_maps = []
    for i in range(NCORES):
        sl = bin_emb[:, i * LC:(i + 1) * LC, :]          # [C, Lc, D]
        embT_i = np.empty((D + 1, C * LC), np.float32)
        embT_i[:D] = sl.transpose(2, 0, 1).reshape(D, C * LC)
        embT_i[D] = 1.0
        in_maps.append({"embT": embT_i, **shared})
    return in_maps


def _assemble(results, cfg=None):
    g = _derived(cfg or DEFAULT_CFG)
    B, C, L, K, NCORES, LC = g["B"], g["C"], g["L"], g["K"], g["NCORES"], g["LC"]
    out = np.empty((B, C, L, K), np.float32)
    for i in range(NCORES):
        r = results[i]["out"]                            # [B, C, K, Lc]
        out[:, :, i * LC:(i + 1) * LC, :] = r.transpose(0, 1, 3, 2)
    return out


def kernel(eos_emb, bin_ids, emb_table, eos_W, eos_b, fc_W, fc_b):
    from concourse.bass_utils import run_bass_kernel_spmd

    if "nc" not in _CACHE:
        _CACHE["nc"] = _build_nc()
    nc = _CACHE["nc"]
    in_maps = _host_prep(eos_emb, bin_ids, emb_table, eos_W, eos_b, fc_W, fc_b)
    res = run_bass_kernel_spmd(nc, in_maps, core_ids=list(range(DEFAULT_CFG["NCORES"])))
    return _assemble(res.results)
